# revision 1
# baseline (speedup 1.0000x reference)
"""LocalAttentionBlock on 8 trn2 cores.

Sharding: 8 cores = 2 batches x 4 sequence blocks of 512 queries.
Each core gets a zero-padded 1024-wide context window (block +/- 256),
transposed on host to [D, s] layout.  All matmuls in float32r.

Per-core pipeline (S^T layout: s on partitions, q on free dim):
  k^T/v^T projections, v re-transposed on PE into v_aug tiles whose
  extra columns hold the per-row validity bit (0 for padded ctx rows)
  replicated 64x -- the AV matmul then emits both the attention
  numerator and a 64-way-replicated softmax denominator in one pass.
  S^T band pieces are packed into two 3-bank psum tensors (all pieces
  >=256 wide for full-rate f32r, none crossing a psum bank); exp on
  ACT with the 1/8 scale folded in; the exact |i-j|<=256 window is cut
  by per-piece affine_selects (predicates absorb the widened regions).
  Normalization: reciprocal on the replica lanes, SBUF->SBUF DMA
  shifts it to the attn lanes, one multiply.  Then y^T = WfT.T@attn+bf.
"""
import sys

import numpy as np

sys.path.insert(0, "/opt/trn_rl_repo")

import concourse.bass as bass  # noqa: E402,F401
import concourse.mybir as mybir  # noqa: E402
import concourse.tile as tile  # noqa: E402
from concourse import bacc  # noqa: E402
from concourse.bass import ts  # noqa: E402
from concourse.bass_utils import run_bass_kernel_spmd  # noqa: E402
from concourse.masks import make_identity  # noqa: E402

F32 = mybir.dt.float32
F32R = mybir.dt.float32r
AF = mybir.ActivationFunctionType
ALU = mybir.AluOpType

B, T, D = 2, 2048, 1024
NH, HD = 16, 64
WIN = 256
BLK = 512      # queries per core
CTX = 1024     # padded context width
NCORES = 8

# S^T band pieces, split across two 3-bank psum tensors A and B.
# (jt, qlo, width, col_offset, bank_first): s-tile jt covers
# s in [128jt, 128jt+128); piece covers q in [qlo, qlo+width).
# Pieces are >=256 wide where possible (f32r matmuls below 256 free
# dim run at 1/4 rate); widened regions are auto-killed by the same
# affine predicates that cut the diagonal.  bank_first pieces carry
# start=True (marks the 2KB zero-region; later pieces in the same
# bank overwrite their own bytes).
PIECES_A = [
    (3, 0, 512, 0, True),
    (4, 0, 512, 512, True),
    (5, 128, 384, 1024, True),
]
PIECES_B = [
    (0, 0, 256, 0, True),
    (1, 0, 256, 256, False),
    (2, 0, 384, 512, True),
    (6, 256, 256, 1024, True),
    (7, 256, 256, 1280, False),
]
# exp width per half: A has a tail pad after 1408; B spans 1536 with a
# dead gap at [896, 1024) (exp of psum garbage there is never read).
EXP_WS = {"exA": 1408, "exB": 1536}
SPW = {"SA": 1536, "SB": 1536}
# Mask windows (col_offset, width, kind, base) per tensor.
# 'lo' keeps sp - y + base >= 0; 'hi' keeps -sp + y + base >= 0.
DIAGS_A = [(384, 128, "lo", 0), (512, 128, "hi", 0), (1024, 128, "hi", 0)]
DIAGS_B = [(0, 256, "lo", 0), (384, 128, "lo", 0), (768, 128, "lo", 0),
           (1024, 128, "hi", 0), (1280, 256, "hi", -128)]


def _build():
    nc = bacc.Bacc(None)
    xT = nc.dram_tensor("xT", [D, CTX], F32R, kind="ExternalInput")
    wqT = nc.dram_tensor("wqT", [D, D], F32R, kind="ExternalInput")
    # cols 0:128 = [Wk | Wk]^T (k duplicated to both partition halves so
    # odd heads can matmul from base_partition 64), cols 128:192 = Wv^T
    wkvT = nc.dram_tensor("wkvT", [D, 192], F32R, kind="ExternalInput")
    wfT = nc.dram_tensor("wfT", [D, D], F32R, kind="ExternalInput")
    bfin = nc.dram_tensor("bfin", [D, 1], F32, kind="ExternalInput")
    valid = nc.dram_tensor("valid", [128, 8], F32, kind="ExternalInput")
    yT = nc.dram_tensor("yT", [D, BLK], F32, kind="ExternalOutput")

    with tile.TileContext(nc) as tc:
        with (
            tc.tile_pool(name="big", bufs=1) as big,
            tc.tile_pool(name="sm", bufs=1) as sm,
            tc.tile_pool(name="smd", bufs=2) as smd,
            tc.tile_pool(name="expp", bufs=2) as expp,
        ):
            # ---- input DMAs (small first; xt before wq before wf) ----
            xt = big.tile([128, 8, CTX], F32R, tag="xt")
            wkv = big.tile([128, 8, 192], F32R, tag="wkv")
            wq = big.tile([128, 8, D], F32R, tag="wq")
            wf = big.tile([128, 8, D], F32R, tag="wf")
            nc.sync.dma_start(
                out=wkv[:, :, 0:128],
                in_=wkvT[:, 0:128].rearrange("(dt p) f -> p dt f", p=128))
            nc.sync.dma_start(out=xt[:, 0, :], in_=xT[ts(0, 128), :])
            nc.sync.dma_start(
                out=wkv[:, :, 128:192],
                in_=wkvT[:, 128:192].rearrange("(dt p) f -> p dt f", p=128))
            for dt in range(1, 8):
                nc.sync.dma_start(out=xt[:, dt, :], in_=xT[ts(dt, 128), :])
            # wq per output-m-tile: q(m) -- and with it the first S heads
            # -- can start as soon as its slice lands
            for m in range(8):
                nc.sync.dma_start(
                    out=wq[:, :, ts(m, 128)],
                    in_=wqT[:, ts(m, 128)].rearrange(
                        "(dt p) c -> p dt c", p=128))
            bf_sb = sm.tile([128, 8], F32, tag="bf")
            nc.sync.dma_start(
                out=bf_sb, in_=bfin.rearrange("(o p) x -> p (o x)", p=128))
            valid_sb = sm.tile([128, 8], F32, tag="valid")
            nc.sync.dma_start(out=valid_sb, in_=valid[:, :])
            nc.sync.dma_start(
                out=wf, in_=wfT.rearrange("(dt p) c -> p dt c", p=128))
            ident = sm.tile([128, 128], F32, tag="ident")
            make_identity(nc, ident)
            ones64 = sm.tile([128, 64], F32, tag="ones64")
            nc.vector.memset(ones64, 1.0)
            # 0/1 triangle constants for DVE-side masking (A half)
            mlo = sm.tile([128, 128], F32, tag="mlo")
            nc.vector.memset(mlo, 1.0)
            nc.gpsimd.affine_select(out=mlo, in_=mlo, compare_op=ALU.is_ge,
                                    fill=0.0, base=0, pattern=[[-1, 128]],
                                    channel_multiplier=1)
            mhi = sm.tile([128, 128], F32, tag="mhi")
            nc.vector.memset(mhi, 1.0)
            nc.gpsimd.affine_select(out=mhi, in_=mhi, compare_op=ALU.is_ge,
                                    fill=0.0, base=0, pattern=[[1, 128]],
                                    channel_multiplier=-1)

            kT2 = big.tile([128, CTX], F32R, tag="kT2")
            vT = big.tile([64, CTX], F32, tag="vT")
            vaug_e = big.tile([128, 8, 128], F32R, tag="vaug_e")
            vaug_o = big.tile([128, 8, 128], F32R, tag="vaug_o")
            qT = big.tile([128, 8, BLK], F32R, tag="qT")
            anrm = big.tile([128, 8, BLK], F32R, tag="anrm")

            # ---- projections (own psum scope, double-buffered) ----
            with tc.tile_pool(name="psproj", bufs=2, space="PSUM") as psp:
                # dt-outer so each xt tile is consumed as it arrives
                kk_ps = [psp.tile([128, 512], F32, tag=f"kk{ch}", bufs=1,
                                  name=f"kk_ps{ch}") for ch in range(2)]
                v_ps = [psp.tile([128, 512], F32, tag=f"vv{ch}", bufs=1,
                                 name=f"v_ps{ch}") for ch in range(2)]
                for dt in range(8):
                    for ch in range(2):
                        nc.tensor.matmul(kk_ps[ch], wkv[:, dt, 0:128],
                                         xt[:, dt, ts(ch, 512)],
                                         start=(dt == 0), stop=(dt == 7))
                        nc.tensor.matmul(v_ps[ch][0:64, :],
                                         wkv[:, dt, 128:192],
                                         xt[:, dt, ts(ch, 512)],
                                         start=(dt == 0), stop=(dt == 7))
                for ch in range(2):
                    nc.scalar.activation(out=kT2[:, ts(ch, 512)],
                                         in_=kk_ps[ch], func=AF.Copy)
                    nc.scalar.activation(out=vT[:, ts(ch, 512)],
                                         in_=v_ps[ch][0:64, :], func=AF.Copy)

                def emit_vaug():
                    # v_aug: [v | valid*64] (even heads) / [valid*64 | v] (odd)
                    for jt in range(8):
                        t_ps = psp.tile([128, 512], F32, tag="tp")
                        nc.tensor.transpose(t_ps[:, 0:64], vT[:, ts(jt, 128)],
                                            ident[0:64, 0:64])
                        nc.vector.tensor_copy(vaug_e[:, jt, 0:64], t_ps[:, 0:64])
                        nc.vector.tensor_copy(vaug_o[:, jt, 64:128],
                                              t_ps[:, 0:64])
                        nc.vector.tensor_scalar_mul(vaug_e[:, jt, 64:128],
                                                    ones64,
                                                    valid_sb[:, jt:jt + 1])
                        nc.vector.tensor_scalar_mul(vaug_o[:, jt, 0:64],
                                                    ones64,
                                                    valid_sb[:, jt:jt + 1])

                for m in range(8):
                    q_ps = psp.tile([128, 512], F32, tag="acc")
                    for dt in range(8):
                        nc.tensor.matmul(q_ps, wq[:, dt, ts(m, 128)],
                                         xt[:, dt, 256:768],
                                         start=(dt == 0), stop=(dt == 7))
                    nc.vector.tensor_copy(qT[:, m, :], q_ps)
                    if m == 0:
                        emit_vaug()

            # ---- attention middle (3+3+2 psum banks) ----
            with (
                tc.tile_pool(name="psSA", bufs=1, space="PSUM") as psA,
                tc.tile_pool(name="psSB", bufs=1, space="PSUM") as psB,
                tc.tile_pool(name="psO", bufs=2, space="PSUM") as psO,
            ):
                def emit_av_norm(h, halves, va):
                    m, r0 = h // 2, 64 * (h % 2)
                    odd = h % 2 == 1
                    o_ps = psO.tile([128, 512], F32, tag="O")
                    nav = 0
                    for (pieces, ex) in halves:
                        for (jt, qlo, w, off, _) in pieces:
                            nc.tensor.matmul(o_ps[:, qlo:qlo + w],
                                             va[:, jt, :],
                                             ex[:, off:off + w],
                                             start=(nav == 0),
                                             stop=(nav == 7),
                                             skip_group_check=True)
                            nav += 1
                    # normalize: denom replicated on the opposite 64
                    # lanes; reciprocal there, gpsimd copy shifts it to
                    # the attn lanes, then one multiply.
                    dlo = 0 if odd else 64
                    rec_a = smd.tile([128, BLK], F32R, tag="rec")
                    with nc.allow_low_precision(reason="denom f32r"):
                        nc.vector.reciprocal(rec_a[dlo:dlo + 64, :],
                                             o_ps[dlo:dlo + 64, :])
                    nc.gpsimd.tensor_copy(rec_a[r0:r0 + 64, :],
                                          rec_a[dlo:dlo + 64, :])
                    nc.vector.tensor_mul(anrm[r0:r0 + 64, m, :],
                                         o_ps[r0:r0 + 64, :],
                                         rec_a[r0:r0 + 64, :])

                pending = None
                for h in range(NH + 1):
                    if h < NH:
                        m, r0 = h // 2, 64 * (h % 2)
                        odd = h % 2 == 1
                        qTh = qT[r0:r0 + 64, m, :]
                        kTh = kT2[r0:r0 + 64, :]
                        va = vaug_o[:, :, :] if odd else vaug_e[:, :, :]
                        halves = []
                        for (pool, pieces, diags, sptag, extag, on_dve) in (
                            (psA, PIECES_A, DIAGS_A, "SA", "exA", True),
                            (psB, PIECES_B, DIAGS_B, "SB", "exB", False),
                        ):
                            s_ps = pool.tile([128, SPW[sptag]], F32, tag=sptag,
                                             name=f"sps{sptag}{h}")
                            for (jt, qlo, w, off, first) in pieces:
                                nc.tensor.matmul(s_ps[:, off:off + w],
                                                 kTh[:, ts(jt, 128)],
                                                 qTh[:, qlo:qlo + w],
                                                 start=first, stop=True,
                                                 skip_group_check=True)
                            xw = EXP_WS[extag]
                            ex = expp.tile([128, xw], F32R, tag=extag)
                            nc.scalar.activation(out=ex,
                                                 in_=s_ps[:, 0:xw],
                                                 func=AF.Exp, scale=0.125)
                            for (doff, dw, kind, dbase) in diags:
                                lo = kind == "lo"
                                if on_dve:
                                    assert dw == 128 and dbase == 0
                                    nc.vector.tensor_mul(
                                        ex[:, doff:doff + dw],
                                        ex[:, doff:doff + dw],
                                        mlo if lo else mhi)
                                else:
                                    nc.gpsimd.affine_select(
                                        out=ex[:, doff:doff + dw],
                                        in_=ex[:, doff:doff + dw],
                                        compare_op=ALU.is_ge,
                                        fill=0.0, base=dbase,
                                        pattern=[[-1 if lo else 1, dw]],
                                        channel_multiplier=1 if lo else -1)
                            halves.append((pieces, ex))
                        cur = (h, halves, va)
                    else:
                        cur = None
                    if pending is not None:
                        emit_av_norm(*pending)
                    pending = cur

                # ---- y^T = wf.T @ attn_norm + bf (alternates the O
                # and q psum slots; no pool-close barrier before Wf) ----
                for o in range(8):
                    y_ps = psO.tile([128, 512], F32, tag="O",
                                    name=f"y_ps{o}")
                    for ft in range(8):
                        nc.tensor.matmul(y_ps, wf[:, ft, ts(o, 128)],
                                         anrm[:, ft, :],
                                         start=(ft == 0), stop=(ft == 7))
                    y_sb = big.tile([128, BLK], F32, tag=f"y{o % 2}",
                                    name=f"y_sb{o}")
                    nc.vector.tensor_scalar_add(y_sb, y_ps,
                                                bf_sb[:, o:o + 1])
                    nc.sync.dma_start(out=yT[ts(o, 128), :], in_=y_sb)

    nc.compile()
    return nc


_NC = None


def _get_nc():
    global _NC
    if _NC is None:
        _NC = _build()
    return _NC


def _prep_inputs(x, Wq, Wk, Wv, Wf, bf):
    x = np.asarray(x, np.float32)
    shared = {
        "wqT": np.ascontiguousarray(np.asarray(Wq, np.float32).T),
        "wkvT": np.ascontiguousarray(
            np.concatenate([np.asarray(Wk, np.float32),
                            np.asarray(Wk, np.float32),
                            np.asarray(Wv, np.float32)], axis=0).T),
        "wfT": np.ascontiguousarray(np.asarray(Wf, np.float32).T),
        "bfin": np.asarray(bf, np.float32).reshape(D, 1),
    }
    in_maps = []
    for c in range(NCORES):
        b, i = divmod(c, 4)
        g0 = 512 * i - WIN  # global position of ctx col 0
        xTc = np.zeros((D, CTX), np.float32)
        lo, hi = max(0, g0), min(T, g0 + CTX)
        xTc[:, lo - g0:hi - g0] = x[b, lo:hi, :].T
        s = np.arange(CTX)
        vmask = ((s + g0 >= 0) & (s + g0 < T)).astype(np.float32)
        in_maps.append({
            "xT": xTc,
            "valid": np.ascontiguousarray(vmask.reshape(8, 128).T),
            **shared,
        })
    return in_maps


def _run(inputs, trace=False):
    nc = _get_nc()
    in_maps = _prep_inputs(**inputs)
    res = run_bass_kernel_spmd(nc, in_maps, core_ids=list(range(NCORES)),
                               trace=trace)
    x = inputs["x"]
    out = np.empty((B, T, D), np.float32)
    for c in range(NCORES):
        b, i = divmod(c, 4)
        out[b, 512 * i:512 * (i + 1), :] = res.results[c]["yT"].T
    return out.astype(np.asarray(x).dtype), res


def kernel(**inputs):
    out, _ = _run(inputs)
    return out



# revision 8
# speedup vs baseline: 1.7802x; 1.7802x over previous
"""LocalAttentionBlock on 8 trn2 cores.

Sharding: 8 cores = 2 batches x 4 sequence blocks of 512 queries.
Each core gets a zero-padded 1024-wide context window (block +/- 256),
transposed on host to [D, s] layout.  All matmuls in bf16 (f32 psum
accumulate) -- f32r runs 2-pass on the PE, bf16 single-pass.

Per-core pipeline (S^T layout: s on partitions, q on free dim):
  One fused [k; v] projection pass (k lanes 0:64, v 64:128); k is
  duplicated to partitions 64:127 by a single SBUF->SBUF DMA so odd
  heads can matmul from base_partition 64.  v re-transposed on PE into
  v_aug tiles whose extra 64 columns hold the per-row validity bit --
  the AV matmul then emits the attention numerator and a 64-way
  replicated softmax denominator in one pass.
  S^T band pieces are trimmed to the exact 128-granular band (2560
  cols/head) and packed gap-free into a 2-bank psum tensor A
  (jt3,jt4) and a 3-bank tensor B (jt2+jt0 | jt1+jt6 | jt5+jt7).
  exp on ACT with the 1/8 scale folded in, bf16 out; the |i-j|<=256
  diagonal is cut by eight 128-wide gpsimd affine_selects per head.
  Normalization: reciprocal_approx_fast on the denom replica lanes,
  SBUF->SBUF DMA shifts it to the attn lanes, one DVE multiply.
  Then y^T = WfT.T @ attn + bf, emitted bf16 and upcast on host.
"""
import sys

import ml_dtypes
import numpy as np

sys.path.insert(0, "/opt/trn_rl_repo")

import concourse.bass as bass  # noqa: E402,F401
import concourse.mybir as mybir  # noqa: E402
import concourse.tile as tile  # noqa: E402
from concourse import bacc  # noqa: E402
from concourse.bass import ts  # noqa: E402
from concourse.bass_utils import run_bass_kernel_spmd  # noqa: E402
from concourse.masks import make_identity  # noqa: E402

F32 = mybir.dt.float32
BF16 = mybir.dt.bfloat16
AF = mybir.ActivationFunctionType
ALU = mybir.AluOpType
BF = ml_dtypes.bfloat16

B, T, D = 2, 2048, 1024
NH, HD = 16, 64
WIN = 256
BLK = 512      # queries per core
CTX = 1024     # padded context width
NCORES = 8

# S^T band pieces (jt, qlo, width, col_offset, start): s-tile jt covers
# s in [128jt, 128jt+128); piece covers q in [qlo, qlo+width).  Trimmed
# to the exact band at 128-col granularity; packed gap-free, no piece
# crossing a 512-col psum bank.  start=True on the first piece of each
# bank.
PIECES_A = [          # 2 banks, 1024 cols
    (3, 0, 512, 0, True),
    (4, 0, 512, 512, True),
]
PIECES_B = [          # 3 banks, 1536 cols
    (2, 0, 384, 0, True),
    (0, 0, 128, 384, False),
    (1, 0, 256, 512, True),
    (6, 256, 256, 768, False),
    (5, 128, 384, 1024, True),
    (7, 384, 128, 1408, False),
]
EXP_WS = {"exA": 1024, "exB": 1536}
SPW = {"SA": 1024, "SB": 1536}
# Diagonal masks (col_offset, kind) per ex tensor; each 128 wide.
# 'lo' keeps sp - y >= 0; 'hi' keeps -sp + y >= 0 (y = col - offset).
DIAGS_A = [(384, "lo"), (512, "hi")]
DIAGS_B = [(256, "lo"), (384, "lo"), (640, "lo"),
           (768, "hi"), (1024, "hi"), (1408, "hi")]


def _build():
    nc = bacc.Bacc(None)
    xT = nc.dram_tensor("xT", [D, CTX], BF16, kind="ExternalInput")
    wqT = nc.dram_tensor("wqT", [D, D], BF16, kind="ExternalInput")
    # cols 0:64 = Wk^T, cols 64:128 = Wv^T
    wkvT = nc.dram_tensor("wkvT", [D, 128], BF16, kind="ExternalInput")
    wfT = nc.dram_tensor("wfT", [D, D], BF16, kind="ExternalInput")
    bfin = nc.dram_tensor("bfin", [D, 1], F32, kind="ExternalInput")
    valid = nc.dram_tensor("valid", [128, 8], F32, kind="ExternalInput")
    yT = nc.dram_tensor("yT", [D, BLK], BF16, kind="ExternalOutput")

    with tile.TileContext(nc) as tc:
        with (
            tc.tile_pool(name="big", bufs=1) as big,
            tc.tile_pool(name="sm", bufs=1) as sm,
            tc.tile_pool(name="recp", bufs=2) as recp,
            tc.tile_pool(name="expp", bufs=2) as expp,
        ):
            # ---- input DMAs (small first; xt before wq before wf) ----
            xt = big.tile([128, 8, CTX], BF16, tag="xt")
            wkv = big.tile([128, 8, 128], BF16, tag="wkv")
            wq = big.tile([128, 8, D], BF16, tag="wq")
            wf = big.tile([128, 8, D], BF16, tag="wf")
            nc.sync.dma_start(
                out=wkv, in_=wkvT.rearrange("(dt p) f -> p dt f", p=128))
            nc.sync.dma_start(out=xt[:, 0, :], in_=xT[ts(0, 128), :])
            for dt in range(1, 8):
                nc.sync.dma_start(out=xt[:, dt, :], in_=xT[ts(dt, 128), :])
            # wq per output-m-tile: q(m) -- and with it the first S heads
            # -- can start as soon as its slice lands
            for m in range(8):
                nc.sync.dma_start(
                    out=wq[:, :, ts(m, 128)],
                    in_=wqT[:, ts(m, 128)].rearrange(
                        "(dt p) c -> p dt c", p=128))
            bf_sb = sm.tile([128, 8], F32, tag="bf")
            nc.sync.dma_start(
                out=bf_sb, in_=bfin.rearrange("(o p) x -> p (o x)", p=128))
            valid_sb = sm.tile([128, 8], F32, tag="valid")
            nc.sync.dma_start(out=valid_sb, in_=valid[:, :])
            nc.sync.dma_start(
                out=wf, in_=wfT.rearrange("(dt p) c -> p dt c", p=128))
            ident = sm.tile([128, 128], BF16, tag="ident")
            make_identity(nc, ident)
            ones64 = sm.tile([128, 64], BF16, tag="ones64")
            nc.vector.memset(ones64, 1.0)

            # kv_sb: rows 0:64 = k^T, rows 64:128 = v^T; khi rows 64:128
            # carry the same k^T so odd heads matmul from base 64.
            kv_sb = big.tile([128, CTX], BF16, tag="kv")
            khi = big.tile([128, CTX], BF16, tag="khi")
            vaug_e = big.tile([128, 8, 128], BF16, tag="vaug_e")
            vaug_o = big.tile([128, 8, 128], BF16, tag="vaug_o")
            qT = big.tile([128, 8, BLK], BF16, tag="qT")
            anrm = big.tile([128, 8, BLK], BF16, tag="anrm")

            # ---- projections (own psum scope, double-buffered) ----
            with tc.tile_pool(name="psproj", bufs=2, space="PSUM") as psp:
                # dt-outer so each xt tile is consumed as it arrives
                kv_ps = [psp.tile([128, 512], F32, tag=f"kv{ch}", bufs=1,
                                  name=f"kv_ps{ch}") for ch in range(2)]
                for dt in range(8):
                    for ch in range(2):
                        nc.tensor.matmul(kv_ps[ch], wkv[:, dt, :],
                                         xt[:, dt, ts(ch, 512)],
                                         start=(dt == 0), stop=(dt == 7))
                for ch in range(2):
                    nc.scalar.activation(out=kv_sb[:, ts(ch, 512)],
                                         in_=kv_ps[ch], func=AF.Copy)
                nc.sync.dma_start(out=khi[64:128, :], in_=kv_sb[0:64, :])

                def emit_vaug():
                    # v_aug: [v | valid*64] (even heads) / [valid*64 | v] (odd)
                    for jt in range(8):
                        t_ps = psp.tile([128, 64], BF16, tag="tp")
                        nc.tensor.transpose(t_ps,
                                            kv_sb[64:128, ts(jt, 128)],
                                            ident[64:128, 64:128])
                        nc.vector.tensor_copy(vaug_e[:, jt, 0:64], t_ps)
                        nc.vector.tensor_copy(vaug_o[:, jt, 64:128], t_ps)
                        nc.vector.tensor_scalar_mul(vaug_e[:, jt, 64:128],
                                                    ones64,
                                                    valid_sb[:, jt:jt + 1])
                        nc.vector.tensor_scalar_mul(vaug_o[:, jt, 0:64],
                                                    ones64,
                                                    valid_sb[:, jt:jt + 1])

                for m in range(8):
                    q_ps = psp.tile([128, 512], F32, tag="acc")
                    for dt in range(8):
                        nc.tensor.matmul(q_ps, wq[:, dt, ts(m, 128)],
                                         xt[:, dt, 256:768],
                                         start=(dt == 0), stop=(dt == 7))
                    nc.vector.tensor_copy(qT[:, m, :], q_ps)
                    if m == 0:
                        emit_vaug()

            # ---- attention middle (2+3+3 psum banks) ----
            with (
                tc.tile_pool(name="psSA", bufs=1, space="PSUM") as psA,
                tc.tile_pool(name="psSB", bufs=1, space="PSUM") as psB,
                tc.tile_pool(name="psO", bufs=3, space="PSUM") as psO,
            ):
                def emit_av_norm(h, halves, va):
                    m, r0 = h // 2, 64 * (h % 2)
                    odd = h % 2 == 1
                    o_ps = psO.tile([128, 512], F32, tag="O")
                    nav = 0
                    for (pieces, ex) in halves:
                        for (jt, qlo, w, off, _) in pieces:
                            nc.tensor.matmul(o_ps[:, qlo:qlo + w],
                                             va[:, jt, :],
                                             ex[:, off:off + w],
                                             start=(nav == 0),
                                             stop=(nav == 7),
                                             skip_group_check=True)
                            nav += 1
                    # normalize: denom replicated on the opposite 64
                    # lanes.  reciprocal_approx_fast only works on SBUF
                    # input at partitions 0:64 on HW, so: ACT-copy the
                    # denom out of psum (lane-locked), route via DMA so
                    # the recip runs on the lower lanes, multiply on the
                    # attn lanes.
                    dlo = 0 if odd else 64
                    den = recp.tile([128, BLK], F32, tag="den")
                    rec = recp.tile([128, BLK], F32, tag="rec")
                    nc.scalar.activation(out=den[dlo:dlo + 64, :],
                                         in_=o_ps[dlo:dlo + 64, :],
                                         func=AF.Copy)
                    if odd:
                        # den already on 0:64: recip there, shift rec up
                        nc.vector.reciprocal_approx_fast(
                            rec[0:64, :], den[0:64, :])
                        nc.sync.dma_start(out=rec[64:128, :],
                                          in_=rec[0:64, :])
                    else:
                        # den on 64:128: shift down, recip on 0:64
                        nc.sync.dma_start(out=den[0:64, :],
                                          in_=den[64:128, :])
                        nc.vector.reciprocal_approx_fast(
                            rec[0:64, :], den[0:64, :])
                    nc.vector.tensor_mul(anrm[r0:r0 + 64, m, :],
                                         o_ps[r0:r0 + 64, :],
                                         rec[r0:r0 + 64, :])

                pending = None
                for h in range(NH + 1):
                    if h < NH:
                        m, r0 = h // 2, 64 * (h % 2)
                        odd = h % 2 == 1
                        qTh = qT[r0:r0 + 64, m, :]
                        kTh = (khi if odd else kv_sb)[r0:r0 + 64, :]
                        va = vaug_o[:, :, :] if odd else vaug_e[:, :, :]
                        halves = []
                        for (pool, pieces, diags, sptag, extag) in (
                            (psA, PIECES_A, DIAGS_A, "SA", "exA"),
                            (psB, PIECES_B, DIAGS_B, "SB", "exB"),
                        ):
                            s_ps = pool.tile([128, SPW[sptag]], F32, tag=sptag,
                                             name=f"sps{sptag}{h}")
                            for (jt, qlo, w, off, first) in pieces:
                                nc.tensor.matmul(s_ps[:, off:off + w],
                                                 kTh[:, ts(jt, 128)],
                                                 qTh[:, qlo:qlo + w],
                                                 start=first, stop=True,
                                                 skip_group_check=True)
                            xw = EXP_WS[extag]
                            ex = expp.tile([128, xw], BF16, tag=extag)
                            nc.scalar.activation(out=ex,
                                                 in_=s_ps[:, 0:xw],
                                                 func=AF.Exp, scale=0.125)
                            for (doff, kind) in diags:
                                lo = kind == "lo"
                                nc.gpsimd.affine_select(
                                    out=ex[:, doff:doff + 128],
                                    in_=ex[:, doff:doff + 128],
                                    compare_op=ALU.is_ge,
                                    fill=0.0, base=0,
                                    pattern=[[-1 if lo else 1, 128]],
                                    channel_multiplier=1 if lo else -1)
                            halves.append((pieces, ex))
                        cur = (h, halves, va)
                    else:
                        cur = None
                    if pending is not None:
                        emit_av_norm(*pending)
                    pending = cur

                # ---- y^T = wf.T @ attn_norm + bf (alternates the O
                # psum slots; no pool-close barrier before Wf) ----
                for o in range(8):
                    y_ps = psO.tile([128, 512], F32, tag="O",
                                    name=f"y_ps{o}")
                    for ft in range(8):
                        nc.tensor.matmul(y_ps, wf[:, ft, ts(o, 128)],
                                         anrm[:, ft, :],
                                         start=(ft == 0), stop=(ft == 7))
                    y_sb = big.tile([128, BLK], BF16, tag=f"y{o % 2}",
                                    name=f"y_sb{o}")
                    nc.vector.tensor_scalar_add(y_sb, y_ps,
                                                bf_sb[:, o:o + 1])
                    nc.sync.dma_start(out=yT[ts(o, 128), :], in_=y_sb)

    nc.compile()
    return nc


_NC = None


def _get_nc():
    global _NC
    if _NC is None:
        _NC = _build()
    return _NC


def _prep_inputs(x, Wq, Wk, Wv, Wf, bf):
    x = np.asarray(x, np.float32)
    shared = {
        "wqT": np.ascontiguousarray(np.asarray(Wq, np.float32).T).astype(BF),
        "wkvT": np.ascontiguousarray(
            np.concatenate([np.asarray(Wk, np.float32),
                            np.asarray(Wv, np.float32)], axis=0).T
        ).astype(BF),
        "wfT": np.ascontiguousarray(np.asarray(Wf, np.float32).T).astype(BF),
        "bfin": np.asarray(bf, np.float32).reshape(D, 1),
    }
    in_maps = []
    for c in range(NCORES):
        b, i = divmod(c, 4)
        g0 = 512 * i - WIN  # global position of ctx col 0
        xTc = np.zeros((D, CTX), np.float32)
        lo, hi = max(0, g0), min(T, g0 + CTX)
        xTc[:, lo - g0:hi - g0] = x[b, lo:hi, :].T
        s = np.arange(CTX)
        vmask = ((s + g0 >= 0) & (s + g0 < T)).astype(np.float32)
        in_maps.append({
            "xT": xTc.astype(BF),
            "valid": np.ascontiguousarray(vmask.reshape(8, 128).T),
            **shared,
        })
    return in_maps


def _run(inputs, trace=False):
    nc = _get_nc()
    in_maps = _prep_inputs(**inputs)
    res = run_bass_kernel_spmd(nc, in_maps, core_ids=list(range(NCORES)),
                               trace=trace)
    x = inputs["x"]
    out = np.empty((B, T, D), np.float32)
    for c in range(NCORES):
        b, i = divmod(c, 4)
        out[b, 512 * i:512 * (i + 1), :] = \
            res.results[c]["yT"].astype(np.float32).T
    return out.astype(np.asarray(x).dtype), res


def kernel(**inputs):
    out, _ = _run(inputs)
    return out
